# revision 6
# baseline (speedup 1.0000x reference)
"""Trainium2 Bass kernel for nn_AlwGAT (GAT-style message passing).

Math (exactly equivalent to the reference, validated to ~6e-7 rel err):
  self = x[:, :36]; others = x[:, 36:].reshape(B, 19, 28)
  att  = softmax_j(others_j . Wa[36:])          # self-part cancels (shift inv.)
  out  = self @ A_self + (sum_j att_j * others_j) @ A_pool + c
where
  A_self = We[:36] @ Wo[:64] + (Ws[:36] + Ws[36:]) @ Wo[64:]
  A_pool = We[36:] @ Wo[:64]
  c      = be @ Wo[:64] + bs @ Wo[64:] + bo     (added on host; zeros here)

Device layout: transposed land.  Per 256-row super-tile:
  PE transposes x -> xT (features on partitions, 5 chunks of <=128)
  PE computes logits l^T = W_L^T @ xT  (weight-mul + group-reduce fused in PE)
  ACT computes e^T = exp(l^T)
  PE broadcasts e across feature partitions via 0/1 selector matmuls, with an
     all-ones block for the 36 self features (yields s = sum_j e_j there)
  DVE does the single elementwise multiply  spT = xT * e_rep
  PE final matmul  out^T = FW^T @ spT  (pooling j-sum folded into contraction)
  PE transposes out^T back, ACT scales by 1/s on PSUM-evacuation, DMA out.
"""

import sys

if "/opt/trn_rl_repo" not in sys.path:
    sys.path.insert(0, "/opt/trn_rl_repo")

import numpy as np

SELF = 36
OTH = 28
J = 19
H = 64
OBS = SELF + OTH * J  # 568
NCORES = 8
BATCH = 65536
ROWS_PER_CORE = BATCH // NCORES  # 8192
TILE_ROWS = 256
NT = ROWS_PER_CORE // TILE_ROWS  # 32
F = [128, 128, 128, 128, 56]  # feature chunk sizes (5 x <=128 covers 568)
NCH = 5

_CACHE = {}


def _build_nc():
    import concourse.bass as bass  # noqa: F401
    import concourse.tile as tile
    from concourse import bacc, mybir
    from concourse.masks import make_identity

    f32 = mybir.dt.float32
    f32r = mybir.dt.float32r

    nc = bacc.Bacc("TRN2", debug=False)
    x_d = nc.dram_tensor("x_in", [ROWS_PER_CORE, OBS], f32, kind="ExternalInput").ap()
    wl_d = nc.dram_tensor("wl_in", [128, NCH, J + 1], f32, kind="ExternalInput").ap()
    b_d = nc.dram_tensor("bsel_in", [J, NCH, 128], f32, kind="ExternalInput").ap()
    fw_d = nc.dram_tensor("fw_in", [128, NCH, H], f32, kind="ExternalInput").ap()
    out_d = nc.dram_tensor("out", [ROWS_PER_CORE, H], f32, kind="ExternalOutput").ap()

    with tile.TileContext(nc) as tc:
        with (
            tc.tile_pool(name="consts", bufs=1) as consts,
            tc.tile_pool(name="xt", bufs=4) as xt_pool,
            tc.tile_pool(name="xts", bufs=2) as xts_pool,
            tc.tile_pool(name="sps", bufs=2) as sp_pool,
            tc.tile_pool(name="small", bufs=3) as small_pool,
            tc.tile_pool(name="psA", bufs=1, space="PSUM") as psA,
            tc.tile_pool(name="psB", bufs=1, space="PSUM") as psB,
            tc.tile_pool(name="psC", bufs=1, space="PSUM") as psC,
            tc.tile_pool(name="psD", bufs=1, space="PSUM") as psD,
        ):
            ident = consts.tile([128, 128], f32)
            make_identity(nc, ident)
            # stage consts as fp32, then round once into f32r tiles (the BIR
            # verifier requires f32r-matmul operands to be *produced* as f32r)
            wl_st = consts.tile([128, NCH, J + 1], f32)
            nc.sync.dma_start(out=wl_st, in_=wl_d)
            wl_sb = consts.tile([128, NCH, J + 1], f32r)
            nc.scalar.copy(out=wl_sb, in_=wl_st)
            b_st = consts.tile([J, NCH, 128], f32)
            nc.sync.dma_start(out=b_st, in_=b_d)
            b_sb = consts.tile([J, NCH, 128], f32r)
            nc.scalar.copy(out=b_sb, in_=b_st)
            fw_st = consts.tile([128, NCH, H], f32)
            nc.sync.dma_start(out=fw_st, in_=fw_d)
            fw_sb = consts.tile([128, NCH, H], f32r)
            nc.scalar.copy(out=fw_sb, in_=fw_st)
            ones_st = consts.tile([J, 1], f32)
            nc.vector.memset(ones_st, 1.0)

            Exp = mybir.ActivationFunctionType.Exp

            for t in range(NT):
                r0 = t * TILE_ROWS
                xts = []
                for h in range(2):
                    xt_h = xt_pool.tile([128, OBS], f32, tag="xt")
                    nc.sync.dma_start(
                        out=xt_h, in_=x_d[r0 + 128 * h : r0 + 128 * (h + 1), :]
                    )
                    xts.append(xt_h)

                # --- transpose x into feature-major chunks (PSUM) ---
                xT_ps = psA.tile([128, 2 * NCH * 128], f32)
                for c in range(NCH):
                    fc = F[c]
                    for h in range(2):
                        nc.tensor.transpose(
                            xT_ps[0:fc, 256 * c + 128 * h : 256 * c + 128 * (h + 1)],
                            xts[h][:, 128 * c : 128 * c + fc],
                            ident,
                        )
                # chunk 4 only covers 56 partitions; evac valid regions only
                xT_sb = xts_pool.tile([128, 2 * NCH * 128], f32r)
                nc.scalar.copy(out=xT_sb[:, 0:1024], in_=xT_ps[:, 0:1024])
                nc.scalar.copy(
                    out=xT_sb[0 : F[4], 1024:1280], in_=xT_ps[0 : F[4], 1024:1280]
                )

                # --- attention logits l^T[j, r] via PE (fused weight+reduce) ---
                lT_ps = psC.tile([128, 256], f32)
                for c in range(NCH):
                    fc = F[c]
                    nc.tensor.matmul(
                        lT_ps[0 : J + 1, 0:256],
                        wl_sb[0:fc, c, :],
                        xT_sb[0:fc, 256 * c : 256 * (c + 1)],
                        start=(c == 0),
                        stop=(c == NCH - 1),
                    )

                # --- e^T = exp(l^T)   (no max-subtraction needed: |l| < ~6) ---
                eT_sb = small_pool.tile([J, 256], f32r, tag="eT")
                nc.scalar.activation(out=eT_sb, in_=lT_ps[0:J, 0:256], func=Exp)

                # --- broadcast e over feature partitions (+ s over self cols) ---
                erep_ps = psB.tile([128, 2 * NCH * 128], f32)
                for c in range(NCH):
                    nc.tensor.matmul(
                        erep_ps[:, 256 * c : 256 * (c + 1)],
                        b_sb[:, c, :],
                        eT_sb,
                        start=True,
                        stop=True,
                    )

                # --- s = sum_j e_j  per row (rows on partitions) ---
                misc_ps = psD.tile([128, 512], f32)
                for h in range(2):
                    nc.tensor.matmul(
                        misc_ps[:, 254 + h : 255 + h],
                        eT_sb[:, 128 * h : 128 * (h + 1)].bitcast(f32),
                        ones_st,
                        start=True,
                        stop=True,
                    )
                r_sb = small_pool.tile([128, 2], f32, tag="r")
                nc.vector.reciprocal(out=r_sb, in_=misc_ps[:, 254:256])

                # --- the one elementwise multiply:  spT = xT * e_rep ---
                sp_sb = sp_pool.tile([128, 2 * NCH * 128], f32r)
                nc.vector.tensor_mul(
                    sp_sb[:, 0:1024], xT_sb[:, 0:1024], erep_ps[:, 0:1024]
                )
                nc.vector.tensor_mul(
                    sp_sb[0 : F[4], 1024:1280],
                    xT_sb[0 : F[4], 1024:1280],
                    erep_ps[0 : F[4], 1024:1280],
                )

                # --- final matmul out^T = FW^T @ spT (pooling sum folded in) ---
                for c in range(NCH):
                    fc = F[c]
                    nc.tensor.matmul(
                        misc_ps[0:H, 256:512],
                        fw_sb[0:fc, c, :],
                        sp_sb[0:fc, 256 * c : 256 * (c + 1)],
                        start=(c == 0),
                        stop=(c == NCH - 1),
                    )
                outT_sb = small_pool.tile([H, 256], f32, tag="outT")
                nc.scalar.copy(out=outT_sb, in_=misc_ps[0:H, 256:512])

                # --- transpose back, scale by 1/s, store ---
                for h in range(2):
                    out_t = misc_ps[:, 256 + 64 * h : 256 + 64 * (h + 1)]
                    nc.tensor.transpose(
                        out_t,
                        outT_sb[:, 128 * h : 128 * (h + 1)],
                        ident[0:H, 0:H],
                    )
                    out_sb = small_pool.tile([128, H], f32, tag="out")
                    nc.scalar.mul(out_sb, out_t, mul=r_sb[:, h : h + 1])
                    nc.sync.dma_start(
                        out=out_d[r0 + 128 * h : r0 + 128 * (h + 1), :], in_=out_sb
                    )

    nc.compile()
    return nc


def _fold_weights(Wa, ba, We, be, Ws, bs, Wo, bo):
    Wa = np.asarray(Wa, np.float64)
    We = np.asarray(We, np.float64)
    Ws = np.asarray(Ws, np.float64)
    Wo = np.asarray(Wo, np.float64)
    wa2 = Wa[SELF:, 0]  # [28]
    A_self = We[:SELF] @ Wo[:H] + (Ws[:SELF] + Ws[SELF:]) @ Wo[H:]  # [36, 64]
    A_pool = We[SELF:] @ Wo[:H]  # [28, 64]
    c = (
        np.asarray(be, np.float64) @ Wo[:H]
        + np.asarray(bs, np.float64) @ Wo[H:]
        + np.asarray(bo, np.float64)
    )  # [64]

    WLp = np.zeros((128, NCH, J + 1), np.float32)  # padded to 20 (fp32r needs even free dim)
    Bp = np.zeros((J, NCH, 128), np.float32)
    FWp = np.zeros((128, NCH, H), np.float32)
    for ch in range(NCH):
        for p in range(128):
            f = 128 * ch + p
            if f >= OBS:
                continue
            if f < SELF:
                Bp[:, ch, p] = 1.0  # ones block -> s for self features
                FWp[p, ch, :] = A_self[f]
            else:
                j0, k = divmod(f - SELF, OTH)
                WLp[p, ch, j0] = wa2[k]
                Bp[j0, ch, p] = 1.0
                FWp[p, ch, :] = A_pool[k]
    return WLp, Bp, FWp, c.astype(np.float32)


def kernel(x, Wa, ba, We, be, Ws, bs, Wo, bo):
    from concourse import bass_utils

    x = np.ascontiguousarray(np.asarray(x, np.float32))
    assert x.shape == (BATCH, OBS), x.shape

    WLp, Bp, FWp, c = _fold_weights(Wa, ba, We, be, Ws, bs, Wo, bo)

    if "nc" not in _CACHE:
        _CACHE["nc"] = _build_nc()
    nc = _CACHE["nc"]

    in_maps = []
    for i in range(NCORES):
        in_maps.append(
            {
                "x_in": x[i * ROWS_PER_CORE : (i + 1) * ROWS_PER_CORE],
                "wl_in": WLp,
                "bsel_in": Bp,
                "fw_in": FWp,
            }
        )

    res = bass_utils.run_bass_kernel_spmd(
        nc,
        in_maps,
        core_ids=list(range(NCORES)),
        trace=_CACHE.get("trace", False),
        **_CACHE.get("run_kwargs", {}),
    )
    _CACHE["last_results"] = res

    out = np.concatenate([np.asarray(res.results[i]["out"]) for i in range(NCORES)], 0)
    if np.any(c):
        out = out + c[None, :]
    return out.astype(np.float32)
